# revision 55
# baseline (speedup 1.0000x reference)
"""DeepSeek-style MoE (32 routed experts, top-8, grouped routing, 2 shared experts)
on 8 Trainium2 NeuronCores via Bass/Tile.

Strategy (expert-parallel, fp8 DoubleRow matmuls with hi/lo error compensation):
- Host computes the routing (sigmoid gate + grouped top-k, bit-matching the
  reference via jax-on-CPU) and gathers each expert's tokens.
- Experts are split into balanced virtual pieces; pieces sorted by size,
  piece of rank r goes to core r%8, slot r//8. All cores run the SAME
  program (slot capacities = per-slot max piece size), so the kernel is
  SPMD-static while the work stays balanced.
- All matmuls run as fp8e4 (e4m3) DoubleRow instructions (2 k-tiles of 128
  per instruction, 0.5 cycles/row = 4x the bf16 FLOP rate). Every operand
  O is decomposed on the host (or on-chip for h) into hi+lo e4m3 parts:
  O/s = Oh + Ol with Ol the quantized residual, so O is represented to
  ~6e-4 relative. A matmul W@x is computed with three terms
      Wh@xh + Wl@xh + Wh@xl      (dropping the ~7e-7 Wl@xl term)
  at 0.75x the bf16-equivalent PE time. k-chunks are paired into the two
  DoubleRow k-tiles, so all operands keep their natural chunk-major layout.
- Per expert-slot MLP on transposed activations:
      gu^T = w_gate_up^T @ x^T     (3-term DR into PSUM, 8 chunk-pairs)
      t    = silu(g * sA)          (ACT, f32)
      hb   = (u * sA*CH) * t       (DVE scalar_tensor_tensor, f32, h*CH)
      hh   = fp8(hb)               (Pool copy)
      hl   = fp8(hb - hh)          (DVE scalar_tensor_tensor)
      y^T  = w_down^T @ h          (3-term DR, PSUM -> ACT copy*sWd/CH -> bf16)
- The shared MLP runs 2-way tensor-parallel (intermediate dim) x 4-way
  data-parallel (tokens), same scheme.
- Host combines in fp32: out = shared partials + 2.5 * topk_w * y rows.

Self-contained: only numpy/jax/ml_dtypes/concourse imports, shapes hardcoded.
"""
import numpy as np
import ml_dtypes

import concourse.bass as bass
import concourse.mybir as mybir
import concourse.tile as tile
from concourse.bass_utils import run_bass_kernel_spmd

F32 = mybir.dt.float32
BF16 = mybir.dt.bfloat16
F8 = mybir.dt.float8e4
E4 = ml_dtypes.float8_e4m3
BF = ml_dtypes.bfloat16
DR = mybir.MatmulPerfMode.DoubleRow
MULT = mybir.AluOpType.mult
ADD = mybir.AluOpType.add

T, H, E, I = 4096, 2048, 32, 1024
TOP_K, N_GROUP, TOPK_GROUP = 8, 8, 4
SI = 2048
ROUTED_SCALING = 2.5
N_CORES = 8
SPLIT_Q = 1152   # experts with more tokens are split into virtual experts
MAX_CHUNK = 1152
# shared expert: SHARED_TP-way split of the intermediate dim x SHARED_DP-way
# split of the tokens (SHARED_TP * SHARED_DP == N_CORES)
SHARED_TP = 2
SHARED_DP = 4
S_TOK = T // SHARED_DP      # tokens per core for the shared MLP
S_SI = SI // SHARED_TP      # intermediate slice per core

_HHC = H // 128   # 16 k-chunks over H
_IC = I // 128    # 8 pairs over I (also h k-chunks)
_SIC = S_SI // 128  # 8 shared pairs

QMAX = 224.0      # e4m3 target max (true max 240; margin for rounding)
CH = 8.0          # h -> fp8 scale (h*CH must stay below ~240)


# ---------------------------------------------------------------- host routing
def _grouped_topk_host(hidden_states, gate_w, gate_bias):
    """Bit-match the reference's jax fp32 routing, on the CPU backend."""
    import jax

    try:
        jax.config.update("jax_platforms", "axon,cpu")
    except Exception:
        pass
    import jax.numpy as jnp

    cpu = jax.devices("cpu")[0]
    with jax.default_device(cpu):
        hs = jnp.asarray(hidden_states)
        gw = jnp.asarray(gate_w)
        bias = jnp.asarray(gate_bias)
        router_logits = hs @ gw
        scores = jax.nn.sigmoid(router_logits)
        sc = scores + bias[None, :]
        t = sc.shape[0]
        g = sc.reshape(t, N_GROUP, E // N_GROUP)
        group_scores = jax.lax.top_k(g, 2)[0].sum(-1)
        grp_idx = jax.lax.top_k(group_scores, TOPK_GROUP)[1]
        grp_mask = jnp.zeros((t, N_GROUP), sc.dtype).at[
            jnp.arange(t)[:, None], grp_idx].set(1.0)
        tok_mask = jnp.repeat(grp_mask, E // N_GROUP, axis=1)
        masked = jnp.where(tok_mask > 0, sc, -jnp.inf)
        topk_ids = jax.lax.top_k(masked, TOP_K)[1]
        w = jnp.take_along_axis(scores, topk_ids, axis=1)
        w = w / (w.sum(-1, keepdims=True) + 1e-20)
        return np.asarray(w), np.asarray(topk_ids)


def _roundup(x, m):
    return -(-x // m) * m


def _chunk_sizes(cap):
    """Split cap (multiple of 8) into chunks <= MAX_CHUNK, multiple of 8."""
    out = []
    rem = cap
    while rem > MAX_CHUNK:
        take = MAX_CHUNK
        out.append(take)
        rem -= take
    out.append(rem)
    return out


def _n_tiles(chunk, cap=512):
    """Split chunk into near-equal matmul N-tiles <= cap (DR moving dim =
    2*nt <= 1024), multiples of 8."""
    k = -(-chunk // cap)
    out = []
    rem = chunk
    for i in range(k, 0, -1):
        nt = -(-rem // i)
        nt = min(rem, -(-nt // 8) * 8)
        out.append(nt)
        rem -= nt
    return out


# ---------------------------------------------------------------- bass program
def _build_nc(caps, sA_r=1.0, sB_r=1.0, sA_s=1.0, sB_s=1.0,
              include_routed=True, include_shared=True):
    """sA_r: silu input scale for routed (sWgu*sX); sB_r: y scale (sWdn/CH);
    sA_s/sB_s: same for the shared MLP."""
    nc = bass.Bass()
    CT = sum(caps)
    n_slots = len(caps)

    # per-slot x tensors: a full-slot load is then one contiguous run per
    # partition (16*cap bytes), dodging the <512B descriptor penalty that a
    # column slice of one big [128, KH, CT] tensor pays for small caps
    xgh_d = [nc.dram_tensor(f"xgh{s}", [128, _HHC, caps[s]], F8,
                            kind="ExternalInput") for s in range(n_slots)]
    xgl_d = [nc.dram_tensor(f"xgl{s}", [128, _HHC, caps[s]], F8,
                            kind="ExternalInput") for s in range(n_slots)]
    # wgu: per (pair, partition, k-chunk): [hi gate|hi up|lo gate|lo up] x 128
    wgu_d = nc.dram_tensor("wgu", [n_slots, _IC, 128, _HHC, 512], F8, kind="ExternalInput")
    # wdn: routed down weights, hi part only (the dl term is dropped for the
    # routed experts: its ~1.4e-2 contribution fits the 2e-2 error budget and
    # saves 1/3 of the phase-B matmul time). Lane-major so 2-m-tile batched
    # loads are contiguous per partition.
    wdn_d = nc.dram_tensor("wdn", [n_slots, 128, _HHC, 1, _IC, 128], F8, kind="ExternalInput")
    sgu_d = nc.dram_tensor("sgu", [_SIC, 128, _HHC, 512], F8, kind="ExternalInput")
    sdn_d = nc.dram_tensor("sdn", [128, _HHC, 2, _SIC, 128], F8, kind="ExternalInput")
    xtsh_d = nc.dram_tensor("xtsh", [128, _HHC, S_TOK], F8, kind="ExternalInput")
    xtsl_d = nc.dram_tensor("xtsl", [128, _HHC, S_TOK], F8, kind="ExternalInput")
    y_d = nc.dram_tensor("y", [128, _HHC, CT], BF16, kind="ExternalOutput")
    ys_d = nc.dram_tensor("ys", [128, _HHC, S_TOK], BF16, kind="ExternalOutput")

    silu = mybir.ActivationFunctionType.Silu
    copy_fn = mybir.ActivationFunctionType.Copy

    from contextlib import ExitStack

    with tile.TileContext(nc) as tc, ExitStack() as ctx:
        xg_pool = ctx.enter_context(tc.tile_pool(name="xgp", bufs=2))
        w_pool = ctx.enter_context(tc.tile_pool(name="wp", bufs=4))
        dn_pool = ctx.enter_context(tc.tile_pool(name="dnp", bufs=7))
        h_pool = ctx.enter_context(tc.tile_pool(name="hp", bufs=2))
        t_pool = ctx.enter_context(tc.tile_pool(name="tp", bufs=4))
        y_pool = ctx.enter_context(tc.tile_pool(name="yp", bufs=4))
        psA = ctx.enter_context(tc.tile_pool(name="psA", bufs=2, space="PSUM"))
        psB = ctx.enter_context(tc.tile_pool(name="psB", bufs=4, space="PSUM"))

        def load_inputs(ch, fast_start=False):
            """Issue chunk input DMAs (wgu pair 0 + x hi/lo) on the SP queue."""
            gu_srcs, _, x_srcs, _, cw, n_pairs, n_kA, _, _, dn_lo = ch
            # routed chunks never read the first xl quarter (x-lo corrections
            # are skipped on k-chunks 0..3), so don't stream it
            q0 = 0 if dn_lo else 1
            wp0 = w_pool.tile([128, n_kA, 512], F8, tag="w", name="wpt")
            xh = xg_pool.tile([128, n_kA, cw], F8, tag="xh", name="xht")
            xl = xg_pool.tile([128, n_kA, cw], F8, tag="xl", name="xlt")
            qk = n_kA // 4
            if fast_start:
                # issue only the first quarter of wgu pair 0 and the first xh
                # chunks first, so the first matmul starts ASAP
                nc.sync.dma_start(out=wp0[:, :qk], in_=gu_srcs[0][:, :qk])
                nc.sync.dma_start(out=xh[:, :qk], in_=x_srcs[0][:, :qk])
                for q in range(1, 4):
                    s = slice(q * qk, (q + 1) * qk)
                    nc.sync.dma_start(out=wp0[:, s], in_=gu_srcs[0][:, s])
                nc.sync.dma_start(out=xh[:, qk:], in_=x_srcs[0][:, qk:])
                nc.sync.dma_start(out=xl[:, q0 * qk:], in_=x_srcs[1][:, q0 * qk:])
            else:
                # small pieces so y stores of the running chunk interleave on
                # the serial DMA fabric
                nc.sync.dma_start(out=wp0[:, :qk * 2], in_=gu_srcs[0][:, :qk * 2])
                nc.sync.dma_start(out=xh[:, :qk], in_=x_srcs[0][:, :qk])
                nc.sync.dma_start(out=wp0[:, qk * 2:], in_=gu_srcs[0][:, qk * 2:])
                for q in range(1, 4):
                    s = slice(q * qk, (q + 1) * qk)
                    nc.sync.dma_start(out=xh[:, s], in_=x_srcs[0][:, s])
                for q in range(q0, 4):
                    s = slice(q * qk, (q + 1) * qk)
                    nc.sync.dma_start(out=xl[:, s], in_=x_srcs[1][:, s])
            return wp0, xh, xl

        def phase_A(ch, loaded, next_ch=None):
            """Gate_up + silu + h-quantize for one chunk (fp8 hi/lo DR).

            ch = (gu_srcs, dn_src, x_srcs, y_dst, cw, n_pairs, n_kA, sA, sB,
                  dn_lo)
            gu_srcs[mp]: DRAM AP [128, n_kA, 512] (hi g|hi u|lo g|lo u of pair)
            dn_src:      DRAM AP [128, _HHC, ndn, n_pairs, 128] (hi[|lo])
            x_srcs[hl]:  DRAM AP [128, n_kA, cw]
            y_dst:       DRAM AP [128, _HHC, cw] (bf16)
            loaded: (wp0, xh, xl) tiles prefetched by the previous chunk.
            Returns the state phase_B needs (h tiles + dn stream queue).
            """
            gu_srcs, dn_src, x_srcs, y_dst, cw, n_pairs, n_kA, sA, sB, dn_lo = ch
            ndn = 2 if dn_lo else 1
            tiles = _n_tiles(cw)
            ncpA = n_kA // 2

            def wp_load(mp):
                wp = w_pool.tile([128, n_kA, 512], F8, tag="w", name="wpt")
                qk = n_kA // 4
                for q in range(4):
                    s = slice(q * qk, (q + 1) * qk)
                    nc.sync.dma_start(out=wp[:, s], in_=gu_srcs[mp][:, s])
                return wp

            wp_next, xh, xl = loaded

            # this chunk's first two dn batches (4 m-tiles) stream in phase A.
            # dn rides the SP queue: the ACT SEQ is the phase-B pacing engine
            # and its 667ns/DMA dispatch slots are needed for the y stores.
            def dn_load(b):
                dt_ = dn_pool.tile([128, 2, ndn, n_pairs, 128], F8, tag="dn", name="dnt")
                nc.sync.dma_start(out=dt_[:], in_=dn_src[:, 2 * b:2 * b + 2])
                return dt_
            dn_q = [dn_load(0), dn_load(1)]

            hh = h_pool.tile([128, n_pairs, cw], F8, tag="hh", name="hht")
            hl = h_pool.tile([128, n_pairs, cw], F8, tag="hl", name="hlt")
            nxt_loaded = None
            for mp in range(n_pairs):
                wp = wp_next
                if mp + 1 < n_pairs:
                    wp_next = wp_load(mp + 1)
                if mp == n_pairs - 2 and next_ch is not None:
                    # next chunk's x + wgu pair 0: issued late in phase A so
                    # they stream during the previous chunk's phase B
                    nxt_loaded = load_inputs(next_ch)
                off = 0
                # the last pair's silu->quantize chain is what the following
                # phase_B's first copies queue behind: finer tiles shrink it
                mp_tiles = _n_tiles(cw, 256)
                for nt in mp_tiles:
                    g = psA.tile([128, 256], F32, tag="g", name="gps")[:, :nt]
                    u = psA.tile([128, 256], F32, tag="u", name="ups")[:, :nt]
                    # xh terms first, xl terms after: the xl tile may still
                    # be streaming when the group starts
                    for cp in range(ncpA):
                        sl = slice(2 * cp, 2 * cp + 2)
                        xhs = xh[:, sl, off:off + nt]
                        st = cp == 0
                        for dst, c0 in ((g, 0), (u, 128)):
                            nc.tensor.matmul(dst, wp[:, sl, c0:c0 + 128], xhs,
                                             start=st, stop=False, perf_mode=DR)
                            nc.tensor.matmul(dst, wp[:, sl, c0 + 256:c0 + 384],
                                             xhs, start=False, stop=False,
                                             perf_mode=DR)
                    # routed chunks skip the x-lo correction on the first 4
                    # k-chunks (2 cps): adds ~1.0e-2 rms to the 1.44e-2 base
                    # (quadrature: 1.76e-2 < 2e-2 gate) and saves 16 cyc/token
                    xl_cp0 = 0 if dn_lo else 2
                    for cp in range(xl_cp0, ncpA):
                        sl = slice(2 * cp, 2 * cp + 2)
                        xls = xl[:, sl, off:off + nt]
                        sp = cp == ncpA - 1
                        for dst, c0 in ((g, 0), (u, 128)):
                            nc.tensor.matmul(dst, wp[:, sl, c0:c0 + 128], xls,
                                             start=False, stop=sp,
                                             perf_mode=DR)
                    t = t_pool.tile([128, 512], F32, tag="t", name="tt")[:, :nt]
                    hb = t_pool.tile([128, 512], F32, tag="hb", name="hbt")[:, :nt]
                    nc.scalar.activation(t, g, silu, scale=sA)
                    # hb = (u * sA*CH) * t = h * CH
                    nc.vector.scalar_tensor_tensor(hb, u, sA * CH, t, MULT, MULT)
                    hhs = hh[:, mp, off:off + nt]
                    hls = hl[:, mp, off:off + nt]
                    nc.gpsimd.tensor_copy(hhs, hb)
                    # hl = hb - hh (quantized residual)
                    nc.vector.scalar_tensor_tensor(hls, hhs, -1.0, hb, MULT, ADD)
                    off += nt
            return (hh, hl, dn_q, dn_load), nxt_loaded

        def phase_B(ch, st, ybatches=(4, 4, 4, 4)):
            """Down matmul + y store for one chunk. Emitted AFTER the next
            chunk's phase_A so the h tiles are long since ready and the
            phase-B engine work overlaps the next chunk's phase-A compute.
            ybatches: m-tiles per y-store DMA (fewer dispatches keep the ACT
            SEQ ahead of the matmul stream; the final chunk tapers so the
            drain ends on a small store)."""
            gu_srcs, dn_src, x_srcs, y_dst, cw, n_pairs, n_kA, sA, sB, dn_lo = ch
            hh, hl, dn_q, dn_load = st
            tiles = _n_tiles(cw)
            ncpB = n_pairs // 2
            cur_dn = None
            yt = None
            yb = list(ybatches)
            m0 = 0
            bs = 0
            eng = 0
            for m in range(_HHC):
                b, mi = divmod(m, 2)
                if mi == 0:
                    cur_dn = dn_q.pop(0)
                    if 2 * b + 4 < _HHC:
                        dn_q.append(dn_load(b + 2))
                if m == m0 + bs:
                    m0, bs = m0 + bs, yb.pop(0)
                    yt = y_pool.tile([128, bs, cw], BF16, tag="y", name="yt")
                dh = cur_dn[:, mi, 0]
                dl = cur_dn[:, mi, 1] if dn_lo else None
                off = 0
                for nt in tiles:
                    py = psB.tile([128, 512], F32, tag="py", name="pyps")[:, :nt]
                    for cp in range(ncpB):
                        sl = slice(2 * cp, 2 * cp + 2)
                        hhs = hh[:, sl, off:off + nt]
                        hls = hl[:, sl, off:off + nt]
                        nc.tensor.matmul(py, dh[:, sl, :], hhs, start=(cp == 0),
                                         stop=False, perf_mode=DR)
                        if dn_lo:
                            nc.tensor.matmul(py, dl[:, sl, :], hhs, start=False,
                                             stop=False, perf_mode=DR)
                        nc.tensor.matmul(py, dh[:, sl, :], hls, start=False,
                                         stop=(cp == ncpB - 1), perf_mode=DR)
                    # PSUM -> bf16 copies alternate ACT/DVE per tile so
                    # neither engine gates the 2-term phase-B matmul stream
                    # and the final drain chain runs on both in parallel
                    if eng % 2:
                        nc.vector.tensor_scalar_mul(yt[:, m - m0, off:off + nt],
                                                    py, sB)
                    else:
                        nc.scalar.activation(yt[:, m - m0, off:off + nt], py,
                                             copy_fn, scale=sB)
                    eng += 1
                    off += nt
                if m == m0 + bs - 1:
                    # Pool queue: its SEQ is idle during phase B, so the
                    # multi-producer waits hoisted onto this store (see
                    # _split_wide_waits) don't block the copy engines' decode
                    nc.gpsimd.dma_start(out=y_dst[:, m0:m0 + bs], in_=yt[:])

        # chunk list: routed slots (ascending cap) then the shared MLP chunk,
        # with the second-smallest routed chunk moved to the end so the final
        # drain (last y stores) is small. The first chunk stays small so the
        # first matmuls start while the bulk of the input stream is in flight.
        chunks = []
        off = 0
        for s in range(len(caps) if include_routed else 0):
            gu_srcs = [wgu_d[s, mp] for mp in range(_IC)]
            so = 0
            for cw in _chunk_sizes(caps[s]):
                o = off
                chunks.append((
                    gu_srcs, wdn_d[s],
                    [xgh_d[s][:, :, so:so + cw], xgl_d[s][:, :, so:so + cw]],
                    y_d[:, :, o:o + cw],
                    cw, _IC, _HHC, sA_r, sB_r, False,
                ))
                off += cw
                so += cw
        if include_shared:
            chunks.append((
                [sgu_d[mp] for mp in range(_SIC)],
                sdn_d[:],
                [xtsh_d[:], xtsl_d[:]],
                ys_d[:],
                S_TOK, _SIC, _HHC, sA_s, sB_s, True,
            ))
        if len(chunks) > 3:
            chunks = chunks[:1] + chunks[2:] + [chunks[1]]

        # PE warmup: dummy matmuls on a zeroed tile while the first input
        # DMAs stream, so the p-state ramp completes before real work arrives
        wu = xg_pool.tile([128, 2, 256], F8, tag="warm", name="wut")
        nc.gpsimd.memset(wu[:], 0)
        wups = psB.tile([128, 512], F32, tag="py", name="pyps")[:, :256]
        loaded = load_inputs(chunks[0], fast_start=True)
        for r in range(96):
            nc.tensor.matmul(wups, wu[:, :, :128], wu[:, :, :256],
                             start=(r == 0), stop=(r == 95), perf_mode=DR)
        # software pipeline: A(0), A(1), B(0), A(2), B(1), ..., B(n-1), B(n).
        # Phase B of chunk i runs on the PE after phase A of chunk i+1, so
        # B's h inputs are always long-ready and B's ACT/DVE/DMA work hides
        # under the next chunk's phase-A matmul stream.
        prev = None
        for i, ch in enumerate(chunks):
            nxt = chunks[i + 1] if i + 1 < len(chunks) else None
            st, loaded = phase_A(ch, loaded, nxt)
            if prev is not None:
                phase_B(*prev)
            prev = (ch, st)
        phase_B(*prev, ybatches=(4, 4, 4, 2, 1, 1))

    _split_wide_waits(nc)
    return nc


# ------------------------------------------------------- walrus wait-limit fix
def _split_wide_waits(nc):
    """walrus codegen allows only 1 sync wait on fused matmuls (and few on
    ctrl ops). Hoist extra waits into single-wait NoOps on the same engine."""
    n = 0
    for f in nc.m.functions:
        for bb in f.blocks:
            il = bb.instructions
            i = 0
            while i < len(il):
                inst = il[i]
                si = inst.sync_info
                waits = list(si.on_wait) if si and si.on_wait else []
                cap = 1
                if len(waits) > cap:
                    inst.sync_info = mybir.SyncInfo(
                        on_wait=waits[:cap], on_update=list(si.on_update or []))
                    nops = [
                        mybir.InstNoOp(
                            name=nc.get_next_instruction_name(),
                            sync_info=mybir.SyncInfo(on_wait=[w], on_update=[]),
                            bass_nofuse=True,
                            engine=inst.engine,
                        )
                        for w in waits[cap:]
                    ]
                    il[i:i] = nops
                    i += len(nops)
                    n += len(nops)
                i += 1
    return n


# ------------------------------------------------------------------- assembly
def plan(topk_w, topk_ids, force_slots=7):
    """Work assignment: split big experts into virtual pieces (<= SPLIT_Q
    tokens), sort pieces by size, piece of rank r -> core r % 8, slot r // 8.
    Slot capacities are the per-slot maxima; slots are ordered smallest-first.
    Returns (caps, assign, tok_of) where assign[s][c] = (expert, start, n)."""
    counts = np.bincount(topk_ids.ravel(), minlength=E)
    tok_of = [np.nonzero(topk_ids == e) for e in range(E)]
    live = [e for e in range(E) if counts[e] > 0]
    cs = [int(counts[e]) for e in live]

    def split_sizes(c, k):
        return [c // k + (1 if i < c % k else 0) for i in range(k)]

    def caps_of(ks):
        sizes = sorted((s for c, k in zip(cs, ks) for s in split_sizes(c, k)),
                       reverse=True)
        ns = -(-len(sizes) // N_CORES)
        return [max(64, _roundup(sizes[N_CORES * s], 8)) for s in range(ns)]

    best_ks, best_cost = None, None
    # 7 slots measured fastest in TimelineSim for this regime (sum of caps
    # 4376 vs 4096 ideal, junctions fully hidden); fall back to a search if
    # it can't hold the live experts.
    if force_slots and force_slots * N_CORES < len(cs):
        force_slots = None
    slot_range = ([force_slots] if force_slots
                  else range(-(-len(cs) // N_CORES), 11))
    for n_slots in slot_range:
        budget = n_slots * N_CORES
        if budget < len(cs):
            continue
        # greedy: split the expert with the largest current piece
        ks = [1] * len(cs)
        while sum(ks) < budget:
            i = max(range(len(cs)), key=lambda a: -(-cs[a] // ks[a]))
            if -(-cs[i] // ks[i]) <= 64:
                break
            ks[i] += 1
        # local search: move splits between experts while it helps.
        # Cost: PE time ~ sum(caps); each slot re-streams full expert weights
        # (junction risk) and caps < 512 pay the 2x small-element DMA penalty.
        def cost(ks):
            cp = caps_of(ks)
            return (sum(cp) + 96 * len(cp)
                    + sum(96 for c in cp if c < 512))
        cur = cost(ks)
        improved = True
        while improved:
            improved = False
            for a in range(len(cs)):
                for b in range(len(cs)):
                    if b == a or ks[b] < 2:
                        continue
                    ks[a] += 1
                    ks[b] -= 1
                    v = cost(ks)
                    if v < cur:
                        cur, improved = v, True
                        break
                    ks[a] -= 1
                    ks[b] += 1
        if any(-(-c // k) > MAX_CHUNK for c, k in zip(cs, ks)):
            # keep every piece within one chunk for SBUF sizing
            pass
        if best_cost is None or cur < best_cost:
            best_cost, best_ks = cur, list(ks)

    ks = best_ks
    pieces = []
    for e, c, k in zip(live, cs, ks):
        st = 0
        for n in split_sizes(c, k):
            pieces.append((e, st, n))
            st += n
    pieces.sort(key=lambda p: -p[2])
    n_slots = -(-len(pieces) // N_CORES)
    pieces += [(0, 0, 0)] * (n_slots * N_CORES - len(pieces))
    slots = [pieces[N_CORES * s:N_CORES * (s + 1)] for s in range(n_slots)]
    slots.sort(key=lambda sl: sl[0][2])  # ascending cap
    caps = [max(64, _roundup(sl[0][2], 8)) for sl in slots]
    return caps, slots, tok_of


def _q8(a):
    """fp32 -> (hi, lo) e4m3 (value ~= hi + lo)."""
    hi = a.astype(E4)
    lo = (a - hi.astype(np.float32)).astype(E4)
    return hi, lo


def _pack_gu(wh8, wl8):
    """[H, 2I'] e4m3 hi+lo (gate cols | up cols) -> [IP, 128, KH, 512]."""
    h, twoi = wh8.shape
    ip = twoi // 256
    kh = h // 128
    out = np.empty((ip, 128, kh, 512), E4)
    for part, w8 in ((0, wh8), (256, wl8)):
        g = w8[:, :ip * 128].reshape(kh, 128, ip, 128).transpose(2, 1, 0, 3)
        u = w8[:, ip * 128:].reshape(kh, 128, ip, 128).transpose(2, 1, 0, 3)
        out[..., part:part + 128] = g
        out[..., part + 128:part + 256] = u
    return out


def _pack_dn(dh8, dl8):
    """[I', H] e4m3 hi+lo -> [128, MH, 2, IC, 128] (lane-major)."""
    i_, h = dh8.shape
    ic = i_ // 128
    mh = h // 128
    out = np.empty((128, mh, 2, ic, 128), E4)
    out[:, :, 0] = dh8.reshape(ic, 128, mh, 128).transpose(1, 2, 0, 3)
    out[:, :, 1] = dl8.reshape(ic, 128, mh, 128).transpose(1, 2, 0, 3)
    return out


def _pack_dn_hi(dh8):
    """[I', H] e4m3 hi only -> [128, MH, 1, IC, 128] (lane-major)."""
    i_, h = dh8.shape
    ic = i_ // 128
    mh = h // 128
    out = np.empty((128, mh, 1, ic, 128), E4)
    out[:, :, 0] = dh8.reshape(ic, 128, mh, 128).transpose(1, 2, 0, 3)
    return out


def _pack_x(x8):
    """[H, n] e4m3 -> [128, KH, n]."""
    h, n = x8.shape
    return np.ascontiguousarray(x8.reshape(h // 128, 128, n).transpose(1, 0, 2))


def kernel(hidden_states, gate_w, gate_bias, w_gate_up, w_down,
           shared_gate_up, shared_down):
    hs = np.ascontiguousarray(hidden_states, dtype=np.float32)
    topk_w, topk_ids = _grouped_topk_host(hs, gate_w, gate_bias)
    caps, slots, tok_of = plan(topk_w, topk_ids)
    n_slots = len(caps)
    CT = sum(caps)
    offs = np.concatenate([[0], np.cumsum(caps)])[:n_slots]

    w_gate_up = np.asarray(w_gate_up, dtype=np.float32)
    w_down = np.asarray(w_down, dtype=np.float32)
    shared_gate_up = np.asarray(shared_gate_up, dtype=np.float32)
    shared_down = np.asarray(shared_down, dtype=np.float32)

    # global e4m3 scales
    sX = float(np.abs(hs).max()) / QMAX
    sWgu = float(np.abs(w_gate_up).max()) / QMAX
    sWdn = float(np.abs(w_down).max()) / QMAX
    sSgu = float(np.abs(shared_gate_up).max()) / QMAX
    sSdn = float(np.abs(shared_down).max()) / QMAX

    xT = np.ascontiguousarray(hs.T) / sX          # [H, T]
    xTh, xTl = _q8(xT)

    # per-expert packed weights (hi/lo merged)
    wgu_packed = []
    wdn_packed = []
    for e in range(E):
        wh, wl = _q8(w_gate_up[e] / sWgu)
        wgu_packed.append(_pack_gu(wh, wl))
        dh, _ = _q8(w_down[e] / sWdn)
        wdn_packed.append(_pack_dn_hi(dh))

    # shared tensors: per TP-slice of the intermediate dim
    SGU_tp, SDN_tp = [], []
    for tp in range(SHARED_TP):
        base = tp * S_SI
        sgu = np.concatenate(
            [shared_gate_up[:, base:base + S_SI],
             shared_gate_up[:, SI + base:SI + base + S_SI]], axis=1) / sSgu
        sh, sl_ = _q8(sgu)
        sdn = shared_down[base:base + S_SI, :] / sSdn
        dh, dl = _q8(sdn)
        SGU_tp.append(_pack_gu(sh, sl_))
        SDN_tp.append(_pack_dn(dh, dl))
    XTS_dp = [
        (np.ascontiguousarray(xTh[:, dp * S_TOK:(dp + 1) * S_TOK]),
         np.ascontiguousarray(xTl[:, dp * S_TOK:(dp + 1) * S_TOK]))
        for dp in range(SHARED_DP)
    ]
    XTS_dp = [(_pack_x(a), _pack_x(b)) for a, b in XTS_dp]

    in_maps = []
    for c in range(N_CORES):
        WGU = np.zeros((n_slots, _IC, 128, _HHC, 512), E4)
        WDN = np.zeros((n_slots, 128, _HHC, 1, _IC, 128), E4)
        tp, dp = c // SHARED_DP, c % SHARED_DP
        im = {
            "wgu": WGU, "wdn": WDN,
            "sgu": SGU_tp[tp], "sdn": SDN_tp[tp],
            "xtsh": XTS_dp[dp][0], "xtsl": XTS_dp[dp][1],
        }
        for s in range(n_slots):
            XGH = np.zeros((128, _HHC, caps[s]), E4)
            XGL = np.zeros((128, _HHC, caps[s]), E4)
            e, st, n = slots[s][c]
            if n > 0:
                idx = tok_of[e][0][st:st + n]
                XGH[:, :, :n] = _pack_x(xTh[:, idx])
                XGL[:, :, :n] = _pack_x(xTl[:, idx])
                WGU[s] = wgu_packed[e]
                WDN[s] = wdn_packed[e]
            im[f"xgh{s}"] = XGH
            im[f"xgl{s}"] = XGL
        in_maps.append(im)

    sA_r = sWgu * sX
    sB_r = sWdn / CH
    sA_s = sSgu * sX
    sB_s = sSdn / CH
    nc = _build_nc(caps, sA_r, sB_r, sA_s, sB_s)
    res = run_bass_kernel_spmd(nc, in_maps, list(range(N_CORES)))

    out = np.zeros((T, H), np.float32)
    for c in range(N_CORES):
        dp = c % SHARED_DP
        ys = res.results[c]["ys"].astype(np.float32)  # [128, 16, S_TOK]
        out[dp * S_TOK:(dp + 1) * S_TOK] += ys.transpose(1, 0, 2).reshape(H, S_TOK).T
    for c in range(N_CORES):
        y = res.results[c]["y"].astype(np.float32)
        y = y.transpose(1, 0, 2).reshape(H, CT)
        for s in range(n_slots):
            e, st, n = slots[s][c]
            if n == 0:
                continue
            idx = tok_of[e][0][st:st + n]
            kpos = tok_of[e][1][st:st + n]
            wts = topk_w[idx, kpos].astype(np.float32) * ROUTED_SCALING
            out[idx] += wts[:, None] * y[:, offs[s]:offs[s] + n].T
    return out

